# revision 16
# baseline (speedup 1.0000x reference)
"""Bidirectional toroidal lattice message passing on 8 Trainium2 cores.

The [N,N] adjacencies are toroidal 3-neighbor shift operators (verified on
host); the 10-step propagation runs fully on-chip. v2 design:

  - partition dim = theta (128); free dims = (dir 2, batch 2, phi 64+2 halo)
  - the REVERSE chain is stored theta-flipped, so both directions use the
    SAME stationary shift matrices P (=T^1) and M (=T^1+I); one 256-wide
    fp32r matmul pair per step (1 cyc/row at >=256 free) replaces four
    128-wide fp32 matmuls (4 cyc/row, double LOW/HIGH pass + 2x LDWEIGHTS)
  - stationaries built on-device (iota+compare) in bf16: 0/1 values exact,
    fast weight load
  - per-step DVE: tmp = psum * g ; x' = c1*x + tmp ; tiny halo refresh
    (g is [128,2,64] broadcast over batch via a 0-stride AP)
  - step accumulation acc += w_s * x' runs on GPSIMD (off critical path)
  - dummy matmuls during the input DMA warm the PE HAM clock gate
  - tail: un-flip reverse acc with a reversal matmul J, combine, 2 DMAs

Batch is sharded 2-per-core across 8 cores; no collectives needed.
"""

import numpy as np

NT, NP, S = 128, 64, 10
N = NT * NP
B = 16
NCORES = 8
BPC = B // NCORES  # batches per core
NH = NP + 2        # phi width incl. wrap halos: [wrap_pre | 0..63 | wrap_post]
NWARM = 8          # HAM warmup matmuls issued during the consts DMA

_FWD = [(1, 0), (0, 1), (1, 1)]
_REV = [(-1, 0), (0, -1), (-1, -1)]


def _diag_vals(adj, shifts):
    idx = np.arange(N)
    ti, pi = idx // NP, idx % NP
    return [adj[idx, ((ti + dt) % NT) * NP + (pi + dp) % NP] for dt, dp in shifts]


def _softmax(x):
    e = np.exp(x - x.max())
    return (e / e.sum()).astype(np.float32)


def _structure_ok(adj, vals):
    for v in vals:
        if np.ptp(v) > 1e-6 * max(1.0, abs(float(v.mean()))):
            return False
    total = adj.sum(dtype=np.float64)
    diag = sum(v.sum(dtype=np.float64) for v in vals)
    return abs(total - diag) < 1e-3


def _reference_fallback(entry, fwd_adj, rev_adj, fwd_sw, fwd_decay, rev_sw,
                        rev_decay, iw, angles):
    # generic dense path (host); only used if the adjacency is not the
    # expected toroidal shift structure.
    def prop(adj, decay, sw):
        d = float(np.clip(decay, 0.5, 0.99))
        af = 0.5 + 0.5 * np.cos(np.abs(angles).mean(axis=1))
        x = entry.astype(np.float32)
        w = _softmax(np.asarray(sw, np.float32))
        acc = np.zeros_like(x)
        for s in range(S):
            p = (x @ adj) * af[None, :]
            x = ((0.3 * x + 0.7 * p) * d).astype(np.float32)
            acc += w[s] * x
        return acc
    f = prop(fwd_adj, fwd_decay, fwd_sw)
    r = prop(rev_adj, rev_decay, rev_sw)
    inter = f * r
    sig = 1.0 / (1.0 + np.exp(-float(iw)))
    return (f + r + np.float32(sig) * inter).astype(np.float32), inter.astype(np.float32)


def _build_program_v2(c1, wst, sig_w):
    """Fused-direction SPMD Bass program (identical on all cores).

    Requires c1 and step weights equal across directions (true for the
    staged model; _host_prep falls back otherwise).

    consts layout (free dim, fp32): [g 2*NP | x0 2*BPC*NH]
    g[t, d, p] = 0.7*decay*v*angle_factor, reverse half theta-flipped.
    x0 is the entry state with phi wrap halos, reverse half theta-flipped.
    """
    import concourse.bacc as bacc
    import concourse.mybir as mybir
    from concourse.bass import AP
    from concourse.tile import TileContext

    fp32 = mybir.dt.float32
    f32r = mybir.dt.float32r
    bf16 = mybir.dt.bfloat16
    i32 = mybir.dt.int32
    OP = mybir.AluOpType

    nc = bacc.Bacc(None, target_bir_lowering=False)

    GE = 2 * NP              # g: [dir, phi]
    XW = 2 * BPC * NH        # x0: [dir, batch, phi+halos]
    CW = GE + XW
    consts_d = nc.dram_tensor("consts", [NT, CW], fp32, kind="ExternalInput")
    out_d = nc.dram_tensor("out_all", [2, BPC, N], fp32, kind="ExternalOutput")

    with TileContext(nc) as tc:
        with (
            tc.tile_pool(name="const", bufs=1) as cpool,
            tc.tile_pool(name="state", bufs=3) as spool,
            tc.tile_pool(name="work", bufs=3) as wpool,
            tc.tile_pool(name="accp", bufs=2) as apool,
            tc.tile_pool(name="psum", bufs=3, space="PSUM") as ppool,
            tc.tile_pool(name="psum1", bufs=1, space="PSUM") as p1pool,
        ):
            consts = cpool.tile([NT, CW], fp32, tag="consts")
            xh = XW // 2
            # three DMA queues so transfers run in parallel
            nc.gpsimd.dma_start(consts[:, 0:GE], consts_d[:, 0:GE])
            nc.scalar.dma_start(consts[:, GE:GE + xh], consts_d[:, GE:GE + xh])
            nc.sync.dma_start(consts[:, GE + xh:CW], consts_d[:, GE + xh:CW])

            # on-device 0/1 shift matrices (shared by both directions since
            # the reverse chain is theta-flipped):
            # vf[k,i] = (i-k) mod 128 ; P = [vf==1] (T^1), M = [vf<2] (T^1+I)
            # vj[k,i] = k+i ; J = [vj==127] (theta reversal, for the tail)
            mats = cpool.tile([NT, 2 * NT], fp32, tag="mats")
            Jt = cpool.tile([NT, NT], fp32, tag="J")
            vf = cpool.tile([NT, NT], i32, tag="vf")
            vj = cpool.tile([NT, NT], i32, tag="vj")
            nc.gpsimd.iota(vf[:], pattern=[[1, NT]], base=NT,
                           channel_multiplier=-1)
            nc.gpsimd.iota(vj[:], pattern=[[1, NT]], base=0,
                           channel_multiplier=1)
            nc.vector.tensor_scalar(vf[:], vf[:], scalar1=NT - 1, scalar2=None,
                                    op0=OP.bitwise_and)
            nc.vector.tensor_scalar(mats[:, 0:NT].bitcast(f32r), vf[:],
                                    scalar1=1, scalar2=None, op0=OP.is_equal)
            nc.vector.tensor_scalar(mats[:, NT:2 * NT].bitcast(f32r), vf[:],
                                    scalar1=2, scalar2=None, op0=OP.is_lt)
            nc.vector.tensor_scalar(Jt[:].bitcast(f32r), vj[:],
                                    scalar1=NT - 1, scalar2=None,
                                    op0=OP.is_equal)
            Pm = mats[:, 0:NT].bitcast(f32r)
            Mm = mats[:, NT:2 * NT].bitcast(f32r)

            # HAM warmup: junk matmuls keep the PE busy while the consts DMA
            # is in flight, so the loop runs at the warm 2.4 GHz clock
            warm_ps = p1pool.tile([NT, 2 * NT], fp32, tag="warm")
            for _ in range(NWARM):
                nc.tensor.matmul(warm_ps[:], Pm, mats[:].bitcast(f32r),
                                 start=True, stop=True)

            g_b = (consts[:, 0:GE].rearrange("t (d p) -> t d p", d=2)
                   .unsqueeze(2).broadcast_to((NT, 2, BPC, NP)))
            # the verifier requires fp32r matmul inputs to come from a
            # rounding-capable producer; DMA is not one, so pass x0 through
            # one DVE copy with an f32r-tagged output
            x0t = spool.tile([NT, 2, BPC, NH], fp32, tag="x")
            x0v = consts[:, GE:CW].rearrange("t (d b p) -> t d b p",
                                             d=2, b=BPC)
            nc.vector.tensor_copy(x0t[:].bitcast(f32r), x0v)
            xs = x0t[:]

            acc = None
            for s in range(S):
                ps = ppool.tile([NT, 2, BPC, NP], fp32, tag="ps")
                xc = xs[:, :, :, 1:NP + 1]
                # M acts on the phi-shifted view: fwd cols 0..63, rev cols
                # 2..65 -- a single AP whose dir stride is 132+2
                xm0 = xs[:, :, :, 0:NP]
                xm = AP(xm0.tensor, xm0.offset,
                        [list(xm0.ap[0]), [BPC * NH + 2, 2], [NH, BPC], [1, NP]])
                nc.tensor.matmul(ps[:], Pm, xc.bitcast(f32r),
                                 start=True, stop=False)
                nc.tensor.matmul(ps[:], Mm, xm.bitcast(f32r),
                                 start=False, stop=True)

                # tmp = psum * g ; x'_center = c1*x + tmp
                tmp = wpool.tile([NT, 2, BPC, NP], fp32, tag="tmp")
                nc.vector.tensor_mul(tmp[:], ps[:], g_b)
                xn = spool.tile([NT, 2, BPC, NH], fp32, tag="x")
                xnc = xn[:, :, :, 1:NP + 1]
                nc.vector.scalar_tensor_tensor(
                    xnc.bitcast(f32r), xs[:, :, :, 1:NP + 1], c1, tmp[:],
                    op0=OP.mult, op1=OP.add)
                if s < S - 1:
                    # refresh wrap halo cols {0,65} from cols {64,1}
                    ho0 = xn[:, :, :, 0:1]
                    ho = AP(ho0.tensor, ho0.offset,
                            [list(ho0.ap[0]), [BPC * NH, 2], [NH, BPC], [NP + 1, 2]])
                    hi = AP(ho0.tensor, ho0.offset + NP,
                            [list(ho0.ap[0]), [BPC * NH, 2], [NH, BPC], [-(NP - 1), 2]])
                    nc.vector.tensor_copy(ho.bitcast(f32r), hi)

                # acc += w_s * x' on GPSIMD (off the critical path)
                an = apool.tile([NT, 2, BPC, NP], fp32, tag="acc")
                # the final acc feeds the fp32r unflip matmul
                an_out = an[:].bitcast(f32r) if s == S - 1 else an[:]
                if acc is None:
                    nc.gpsimd.tensor_scalar_mul(an_out, xnc, wst[s])
                else:
                    # Pool rejects the fused scalar_tensor_tensor; two ops
                    tw = apool.tile([NT, 2, BPC, NP], fp32, tag="tw")
                    nc.gpsimd.tensor_scalar_mul(tw[:], xnc, wst[s])
                    nc.gpsimd.tensor_add(an_out, tw[:], acc[:])
                acc = an
                xs = xn[:]

            # tail: unflip reverse acc (J @ acc_r), combine, store
            f = acc[:, 0]
            rF = acc[:, 1]
            ps_r = p1pool.tile([NT, BPC, NP], fp32, tag="psr")
            nc.tensor.matmul(ps_r[:], Jt[:].bitcast(f32r), rF.bitcast(f32r),
                             start=True, stop=True)
            out2 = wpool.tile([NT, 2, BPC, NP], fp32, tag="out2")
            inter = out2[:, 1]
            nc.vector.tensor_mul(inter, f, ps_r[:])
            fr = wpool.tile([NT, BPC, NP], fp32, tag="fr")
            nc.vector.tensor_add(fr[:], f, ps_r[:])
            ov = out_d[:].rearrange("o b (t p) -> o t b p", t=NT)
            nc.scalar.dma_start(ov[1], inter)
            nc.vector.scalar_tensor_tensor(
                out2[:, 0], inter, sig_w, fr[:], op0=OP.mult, op1=OP.add)
            nc.sync.dma_start(ov[0], out2[:, 0])

    nc.finalize()
    return nc


def _build_program_v1(c1, w, sig_w):
    """Per-direction fallback program (handles c1f != c1r or wf != wr)."""
    import concourse.bacc as bacc
    import concourse.mybir as mybir
    from concourse.tile import TileContext

    fp32 = mybir.dt.float32
    i32 = mybir.dt.int32
    mm_dt = fp32
    OP = mybir.AluOpType

    nc = bacc.Bacc(None, target_bir_lowering=False)

    GE = 2 * BPC * NP
    XW = BPC * NH
    CW = GE + XW
    consts_d = nc.dram_tensor("consts", [NT, CW], fp32, kind="ExternalInput")
    out_d = nc.dram_tensor("out_all", [2, BPC, N], fp32, kind="ExternalOutput")

    (c1f, c1r), (wf, wr) = c1, w

    g_off = 0
    x0_off = g_off + GE

    with TileContext(nc) as tc:
        with (
            tc.tile_pool(name="const", bufs=1) as cpool,
            tc.tile_pool(name="state", bufs=3) as spool,
            tc.tile_pool(name="work", bufs=3) as wpool,
            tc.tile_pool(name="accp", bufs=2) as apool,
            tc.tile_pool(name="psum", bufs=4, space="PSUM") as ppool,
        ):
            consts = cpool.tile([NT, CW], fp32, tag="consts")
            xh = XW // 2
            nc.scalar.dma_start(consts[:, x0_off:x0_off + xh],
                                consts_d[:, x0_off:x0_off + xh])
            nc.sync.dma_start(consts[:, x0_off + xh:x0_off + XW],
                              consts_d[:, x0_off + xh:x0_off + XW])
            nc.sync.dma_start(consts[:, g_off:g_off + GE],
                              consts_d[:, g_off:g_off + GE])

            matsf = cpool.tile([NT, 4 * NT], fp32, tag="matsf")
            vf = cpool.tile([NT, NT], i32, tag="vf")
            vr = cpool.tile([NT, NT], i32, tag="vr")
            nc.gpsimd.iota(vf[:], pattern=[[1, NT]], base=NT,
                           channel_multiplier=-1)
            nc.gpsimd.iota(vr[:], pattern=[[-1, NT]], base=NT,
                           channel_multiplier=1)
            nc.vector.tensor_scalar(vf[:], vf[:], scalar1=NT - 1, scalar2=None,
                                    op0=OP.bitwise_and)
            nc.vector.tensor_scalar(vr[:], vr[:], scalar1=NT - 1, scalar2=None,
                                    op0=OP.bitwise_and)
            nc.vector.tensor_scalar(matsf[:, 0:NT], vf[:], scalar1=1,
                                    scalar2=None, op0=OP.is_equal)
            nc.vector.tensor_scalar(matsf[:, 2 * NT:3 * NT], vr[:], scalar1=1,
                                    scalar2=None, op0=OP.is_equal)
            nc.vector.tensor_scalar(matsf[:, NT:2 * NT], vf[:], scalar1=2,
                                    scalar2=None, op0=OP.is_lt)
            nc.vector.tensor_scalar(matsf[:, 3 * NT:4 * NT], vr[:], scalar1=2,
                                    scalar2=None, op0=OP.is_lt)
            pm = {k: matsf[:, i * NT:(i + 1) * NT].bitcast(mm_dt)
                  for i, k in enumerate(("Pf", "Mf", "Pr", "Mr"))}
            g_e = consts[:, g_off:g_off + GE].rearrange(
                "t (d b p) -> t d b p", d=2, b=BPC)

            x0 = consts[:, x0_off:x0_off + XW].rearrange(
                "t (b p) -> t b p", b=BPC)

            DIRS = (
                dict(d=0, P="Pf", M="Mf", c1=c1f, w=wf, xtag="xf"),
                dict(d=1, P="Pr", M="Mr", c1=c1r, w=wr, xtag="xr"),
            )
            xs = [x0, x0]
            accs = [None, None]
            for s in range(S):
                xcs = [None, None]
                for dd in DIRS:
                    d = dd["d"]
                    xd = xs[d]
                    ps = ppool.tile([NT, BPC, NP], fp32, tag=f"ps{d}")
                    xm = xd.bitcast(mm_dt)
                    lo = 0 if d == 0 else 2
                    nc.tensor.matmul(ps[:], pm[dd["P"]], xm[:, :, 1:NP + 1],
                                     start=True, stop=False)
                    nc.tensor.matmul(ps[:], pm[dd["M"]], xm[:, :, lo:lo + NP],
                                     start=False, stop=True)

                    tmp = wpool.tile([NT, BPC, NP], fp32, tag=f"tmp{d}")
                    nc.vector.tensor_mul(tmp[:], ps[:], g_e[:, d])
                    xn = spool.tile([NT, BPC, NH], fp32, tag=dd["xtag"])
                    xcs[d] = xn[:, :, 1:NP + 1]
                    nc.vector.scalar_tensor_tensor(
                        xcs[d], xd[:, :, 1:NP + 1], dd["c1"], tmp[:],
                        op0=OP.mult, op1=OP.add)
                    if s < S - 1:
                        nc.gpsimd.tensor_copy(xn[:, :, 0:1], xn[:, :, NP:NP + 1])
                        nc.gpsimd.tensor_copy(xn[:, :, NP + 1:NP + 2], xn[:, :, 1:2])
                    xs[d] = xn[:]

                for dd in DIRS:
                    d = dd["d"]
                    an = apool.tile([NT, BPC, NP], fp32, tag=f"acc{d}")
                    if accs[d] is None:
                        nc.vector.tensor_scalar_mul(an[:], xcs[d], dd["w"][0])
                    else:
                        nc.vector.scalar_tensor_tensor(
                            an[:], xcs[d], dd["w"][s], accs[d][:],
                            op0=OP.mult, op1=OP.add)
                    accs[d] = an
            accs = [accs[0][:], accs[1][:]]

            out2 = wpool.tile([NT, 2, BPC, NP], fp32, tag="out2")
            inter = out2[:, 1]
            nc.vector.tensor_mul(inter, accs[0], accs[1])
            fr = wpool.tile([NT, BPC, NP], fp32, tag="fr")
            nc.vector.tensor_add(fr[:], accs[0], accs[1])
            ov = out_d[:].rearrange("o b (t p) -> o t b p", t=NT)
            nc.scalar.dma_start(ov[1], inter)
            nc.vector.scalar_tensor_tensor(
                out2[:, 0], inter, sig_w, fr[:], op0=OP.mult, op1=OP.add)
            nc.sync.dma_start(ov[0], out2[:, 0])

    nc.finalize()
    return nc


def _host_prep(inputs):
    entry = np.ascontiguousarray(np.asarray(inputs["entry_probs"], np.float32))
    fwd_adj = np.asarray(inputs["forward_adj"], np.float32)
    rev_adj = np.asarray(inputs["reverse_adj"], np.float32)
    angles = np.asarray(inputs["bounce_angles"], np.float32)

    vf = _diag_vals(fwd_adj, _FWD)
    vr = _diag_vals(rev_adj, _REV)
    ok = _structure_ok(fwd_adj, vf) and _structure_ok(rev_adj, vr)

    df = float(np.clip(float(np.asarray(inputs["forward_decay"])), 0.5, 0.99))
    dr = float(np.clip(float(np.asarray(inputs["reverse_decay"])), 0.5, 0.99))
    wf = _softmax(np.asarray(inputs["forward_step_weights"], np.float32))
    wr = _softmax(np.asarray(inputs["reverse_step_weights"], np.float32))
    sig = float(1.0 / (1.0 + np.exp(-float(np.asarray(inputs["interaction_weight"])))))

    vbf = [float(v.mean()) for v in vf]   # [v10, v01, v11]
    vbr = [float(v.mean()) for v in vr]
    # 0/1 matrices require one shared constant per direction
    for vs in (vbf, vbr):
        if abs(vs[0] - vs[1]) > 1e-6 * abs(vs[0]) or \
           abs(vs[0] - vs[2]) > 1e-6 * abs(vs[0]):
            ok = False

    af = (0.5 + 0.5 * np.cos(np.abs(angles).mean(axis=1))).astype(np.float32)
    af2 = af.reshape(NT, NP)

    c1f, c1r = 0.3 * df, 0.3 * dr
    fused = ok and abs(c1f - c1r) < 1e-12 and \
        all(abs(a - b) < 1e-12 for a, b in zip(wf, wr))

    consts_list = []
    if fused:
        # g[t, d, p]; reverse half theta-flipped
        g = np.empty((NT, 2, NP), np.float32)
        g[:, 0] = (0.7 * df * vbf[0]) * af2
        g[:, 1] = (0.7 * dr * vbr[0]) * af2[::-1]
        for c in range(NCORES):
            e = entry[c * BPC:(c + 1) * BPC].reshape(BPC, NT, NP)
            x0 = np.empty((NT, 2, BPC, NH), np.float32)
            x0[:, 0, :, 1:NP + 1] = e.transpose(1, 0, 2)
            x0[:, 1, :, 1:NP + 1] = e[:, ::-1].transpose(1, 0, 2)
            x0[:, :, :, 0] = x0[:, :, :, NP]
            x0[:, :, :, NP + 1] = x0[:, :, :, 1]
            consts_list.append(np.ascontiguousarray(np.concatenate(
                [g.reshape(NT, -1), x0.reshape(NT, -1)], axis=1)))
    elif ok:
        g_e = np.empty((NT, 2, BPC, NP), np.float32)
        g_e[:, 0] = (0.7 * df * vbf[0]) * af2[:, None, :]
        g_e[:, 1] = (0.7 * dr * vbr[0]) * af2[:, None, :]
        consts_common = g_e.reshape(NT, -1)
        for c in range(NCORES):
            e = entry[c * BPC:(c + 1) * BPC].reshape(BPC, NT, NP).transpose(1, 0, 2)
            x0 = np.empty((NT, BPC, NH), np.float32)
            x0[:, :, 1:NP + 1] = e
            x0[:, :, 0] = e[:, :, NP - 1]
            x0[:, :, NP + 1] = e[:, :, 0]
            consts_list.append(np.ascontiguousarray(
                np.concatenate([consts_common, x0.reshape(NT, -1)], axis=1)))

    meta = dict(
        ok=ok, fused=fused,
        c1=(c1f, c1r),
        w=(list(map(float, wf)), list(map(float, wr))),
        sig=sig, consts_list=consts_list,
    )
    return meta


_PROGRAM_CACHE = {}
LAST_RESULT = None


def kernel(**inputs):
    meta = _host_prep(inputs)
    if not meta["ok"]:
        return _reference_fallback(
            np.asarray(inputs["entry_probs"], np.float32),
            np.asarray(inputs["forward_adj"], np.float32),
            np.asarray(inputs["reverse_adj"], np.float32),
            inputs["forward_step_weights"], inputs["forward_decay"],
            inputs["reverse_step_weights"], inputs["reverse_decay"],
            inputs["interaction_weight"], np.asarray(inputs["bounce_angles"], np.float32))

    # If tracing is requested via BASS_TRACE but the image's antenv lacks
    # axon_hooks, provide the hook so run_bass_kernel_spmd doesn't crash.
    import os as _os
    if _os.environ.get("BASS_TRACE"):
        try:
            import antenv.axon_hooks  # noqa: F401
        except ImportError:
            try:
                import sys as _sys
                import types as _types
                import trn_agent_boot.trn_boot as _tb
                _hook = _tb._ntff_profile_via_ctypes("/opt/axon/libaxon_pjrt.so")
                _mod = _types.ModuleType("antenv.axon_hooks")
                _mod.get_axon_ntff_profile_hook = lambda: _hook
                _mod.set_axon_ntff_profile_hook = lambda h: None
                _sys.modules["antenv.axon_hooks"] = _mod
            except Exception:
                _os.environ.pop("BASS_TRACE", None)

    from concourse import bass_utils

    key = (tuple(meta["c1"]), tuple(meta["w"][0]), tuple(meta["w"][1]),
           meta["sig"], meta["fused"])
    if key not in _PROGRAM_CACHE:
        if meta["fused"]:
            _PROGRAM_CACHE[key] = _build_program_v2(
                meta["c1"][0], meta["w"][0], meta["sig"])
        else:
            _PROGRAM_CACHE[key] = _build_program_v1(
                meta["c1"], meta["w"], meta["sig"])
    nc = _PROGRAM_CACHE[key]

    in_maps = [{"consts": meta["consts_list"][c]} for c in range(NCORES)]
    res = bass_utils.run_bass_kernel_spmd(nc, in_maps, core_ids=list(range(NCORES)))
    global LAST_RESULT
    LAST_RESULT = res

    combined = np.concatenate([r["out_all"][0] for r in res.results], axis=0)
    interaction = np.concatenate([r["out_all"][1] for r in res.results], axis=0)
    return combined, interaction


# revision 29
# speedup vs baseline: 1.9696x; 1.9696x over previous
"""Bidirectional toroidal lattice message passing on 8 Trainium2 cores.

The [N,N] adjacencies are toroidal 3-neighbor shift operators (verified on
host); the 10-step propagation runs fully on-chip. v2 design:

  - partition dim = theta (128); free dims = (dir 2, batch 2, phi 64+2 halo)
  - the REVERSE chain is stored theta-flipped, so both directions use the
    SAME stationary shift matrices P (=T^1) and M (=T^1+I); one 256-wide
    fp32r matmul pair per step (1 cyc/row at >=256 free) replaces four
    128-wide fp32 matmuls (4 cyc/row, double LOW/HIGH pass + 2x LDWEIGHTS)
  - stationaries built on-device (iota+compare) in bf16: 0/1 values exact,
    fast weight load
  - per-step DVE: tmp = psum * g ; x' = c1*x + tmp ; tiny halo refresh
    (g is [128,2,64] broadcast over batch via a 0-stride AP)
  - step accumulation acc += w_s * x' runs on GPSIMD (off critical path)
  - dummy matmuls during the input DMA warm the PE HAM clock gate
  - tail: un-flip reverse acc with a reversal matmul J, combine, 2 DMAs

Batch is sharded 2-per-core across 8 cores; no collectives needed.
"""

import numpy as np

NT, NP, S = 128, 64, 10
N = NT * NP
B = 16
NCORES = 8
BPC = B // NCORES  # batches per core
NH = NP + 2        # phi width incl. wrap halos: [wrap_pre | 0..63 | wrap_post]
NWARM = 8          # HAM warmup matmuls issued during the consts DMA

_FWD = [(1, 0), (0, 1), (1, 1)]
_REV = [(-1, 0), (0, -1), (-1, -1)]


def _diag_vals(adj, shifts):
    idx = np.arange(N)
    ti, pi = idx // NP, idx % NP
    return [adj[idx, ((ti + dt) % NT) * NP + (pi + dp) % NP] for dt, dp in shifts]


def _softmax(x):
    e = np.exp(x - x.max())
    return (e / e.sum()).astype(np.float32)


def _structure_ok(adj, vals):
    for v in vals:
        if np.ptp(v) > 1e-6 * max(1.0, abs(float(v.mean()))):
            return False
    total = adj.sum(dtype=np.float64)
    diag = sum(v.sum(dtype=np.float64) for v in vals)
    return abs(total - diag) < 1e-3


def _reference_fallback(entry, fwd_adj, rev_adj, fwd_sw, fwd_decay, rev_sw,
                        rev_decay, iw, angles):
    # generic dense path (host); only used if the adjacency is not the
    # expected toroidal shift structure.
    def prop(adj, decay, sw):
        d = float(np.clip(decay, 0.5, 0.99))
        af = 0.5 + 0.5 * np.cos(np.abs(angles).mean(axis=1))
        x = entry.astype(np.float32)
        w = _softmax(np.asarray(sw, np.float32))
        acc = np.zeros_like(x)
        for s in range(S):
            p = (x @ adj) * af[None, :]
            x = ((0.3 * x + 0.7 * p) * d).astype(np.float32)
            acc += w[s] * x
        return acc
    f = prop(fwd_adj, fwd_decay, fwd_sw)
    r = prop(rev_adj, rev_decay, rev_sw)
    inter = f * r
    sig = 1.0 / (1.0 + np.exp(-float(iw)))
    return (f + r + np.float32(sig) * inter).astype(np.float32), inter.astype(np.float32)


def _build_program_v2(c1, wst, sig_w):
    """Fused-direction SPMD Bass program (identical on all cores).

    Requires c1 and step weights equal across directions (true for the
    staged model; _host_prep falls back otherwise).

    consts layout (free dim, fp32): [g 2*NP | x0 2*BPC*NH]
    g[t, d, p] = 0.7*decay*v*angle_factor, reverse half theta-flipped.
    x0 is the entry state with phi wrap halos, reverse half theta-flipped.
    """
    import concourse.bacc as bacc
    import concourse.mybir as mybir
    from concourse.bass import AP
    from concourse.tile import TileContext

    fp32 = mybir.dt.float32
    fp16 = mybir.dt.float16
    i32 = mybir.dt.int32
    OP = mybir.AluOpType

    nc = bacc.Bacc(None, target_bir_lowering=False)

    GE = 2 * NP              # g: [dir, phi]
    XW = 2 * BPC * NH        # x0: [dir, batch, phi+halos]
    consts_d = nc.dram_tensor("consts", [NT, GE], fp32, kind="ExternalInput")
    x0_d = nc.dram_tensor("x0", [NT, XW], fp16, kind="ExternalInput")
    out_d = nc.dram_tensor("out_all", [2, BPC, N], fp32, kind="ExternalOutput")

    with TileContext(nc) as tc:
        with (
            tc.tile_pool(name="const", bufs=1) as cpool,
            tc.tile_pool(name="state", bufs=3) as spool,
            tc.tile_pool(name="work", bufs=3) as wpool,
            tc.tile_pool(name="accp", bufs=2) as apool,
            tc.tile_pool(name="psum", bufs=3, space="PSUM") as ppool,
            tc.tile_pool(name="psum1", bufs=1, space="PSUM") as p1pool,
        ):
            consts = cpool.tile([NT, GE], fp32, tag="consts")
            x0t = spool.tile([NT, 2, BPC, NH], fp16, tag="x")
            x0f = x0t[:].rearrange("t d b p -> t (d b p)")
            xh = XW // 2
            # three DMA queues so transfers run in parallel
            nc.gpsimd.dma_start(consts[:], consts_d[:])
            nc.scalar.dma_start(x0f[:, 0:xh], x0_d[:, 0:xh])
            nc.sync.dma_start(x0f[:, xh:XW], x0_d[:, xh:XW])

            # on-device 0/1 shift matrices (shared by both directions since
            # the reverse chain is theta-flipped):
            # vf[k,i] = (i-k) mod 128 ; P = [vf==1] (T^1), M = [vf<2] (T^1+I)
            # vj[k,i] = k+i ; J = [vj==127] (theta reversal, for the tail)
            mats = cpool.tile([NT, 2 * NT], fp16, tag="mats")
            Jt = cpool.tile([NT, NT], fp16, tag="J")
            vf = cpool.tile([NT, NT], i32, tag="vf")
            vj = cpool.tile([NT, NT], i32, tag="vj")
            nc.gpsimd.iota(vf[:], pattern=[[1, NT]], base=NT,
                           channel_multiplier=-1)
            nc.gpsimd.iota(vj[:], pattern=[[1, NT]], base=0,
                           channel_multiplier=1)
            nc.vector.tensor_scalar(vf[:], vf[:], scalar1=NT - 1, scalar2=None,
                                    op0=OP.bitwise_and)
            nc.vector.tensor_scalar(mats[:, 0:NT], vf[:],
                                    scalar1=1, scalar2=None, op0=OP.is_equal)
            nc.vector.tensor_scalar(mats[:, NT:2 * NT], vf[:],
                                    scalar1=2, scalar2=None, op0=OP.is_lt)
            nc.vector.tensor_scalar(Jt[:], vj[:],
                                    scalar1=NT - 1, scalar2=None,
                                    op0=OP.is_equal)
            Pm = mats[:, 0:NT]
            Mm = mats[:, NT:2 * NT]

            # HAM warmup: junk matmuls keep the PE busy while the consts DMA
            # is in flight, so the loop runs at the warm 2.4 GHz clock
            warm_ps = p1pool.tile([NT, 2 * NT], fp32, tag="warm")
            for _ in range(NWARM):
                nc.tensor.matmul(warm_ps[:], Pm, mats[:], start=True, stop=True)

            g_b = (consts[:].rearrange("t (d p) -> t d p", d=2)
                   .unsqueeze(2).broadcast_to((NT, 2, BPC, NP)))
            xs = x0t[:]

            acc = None
            for s in range(S):
                ps = ppool.tile([NT, 2, BPC, NP], fp32, tag="ps")
                xc = xs[:, :, :, 1:NP + 1]
                # M acts on the phi-shifted view: fwd cols 0..63, rev cols
                # 2..65 -- a single AP whose dir stride is 132+2
                xm0 = xs[:, :, :, 0:NP]
                xm = AP(xm0.tensor, xm0.offset,
                        [list(xm0.ap[0]), [BPC * NH + 2, 2], [NH, BPC], [1, NP]])
                nc.tensor.matmul(ps[:], Pm, xc, start=True, stop=False)
                nc.tensor.matmul(ps[:], Mm, xm, start=False, stop=True)

                # tmp = psum * g ; x'_center = c1*x + tmp
                tmp = wpool.tile([NT, 2, BPC, NP], fp32, tag="tmp")
                nc.vector.tensor_mul(tmp[:], ps[:], g_b)
                xn = spool.tile([NT, 2, BPC, NH], fp16, tag="x")
                xnc = xn[:, :, :, 1:NP + 1]
                nc.vector.scalar_tensor_tensor(
                    xnc, xs[:, :, :, 1:NP + 1], c1, tmp[:],
                    op0=OP.mult, op1=OP.add)
                if s < S - 1:
                    # refresh wrap halo cols {0,65} from cols {64,1}
                    ho0 = xn[:, :, :, 0:1]
                    ho = AP(ho0.tensor, ho0.offset,
                            [list(ho0.ap[0]), [BPC * NH, 2], [NH, BPC], [NP + 1, 2]])
                    hi = AP(ho0.tensor, ho0.offset + NP,
                            [list(ho0.ap[0]), [BPC * NH, 2], [NH, BPC], [-(NP - 1), 2]])
                    nc.vector.tensor_copy(ho, hi)

                # acc += w_s * x' on GPSIMD (off the critical path)
                # acc += w_s * x' on DVE, scheduled in the gap while the PE
                # runs the next step's matmuls; the final acc is fp16 so the
                # unflip matmul can consume it
                if s == S - 1:
                    an = apool.tile([NT, 2, BPC, NP], fp16, tag="acch")
                else:
                    an = apool.tile([NT, 2, BPC, NP], fp32, tag="acc")
                if acc is None:
                    nc.vector.tensor_scalar_mul(an[:], xnc, wst[s])
                else:
                    nc.vector.scalar_tensor_tensor(
                        an[:], xnc, wst[s], acc[:], op0=OP.mult, op1=OP.add)
                acc = an
                xs = xn[:]

            # tail: unflip reverse acc (J @ acc_r), combine, store
            f = acc[:, 0]
            rF = acc[:, 1]
            ps_r = p1pool.tile([NT, BPC, NP], fp32, tag="psr")
            nc.tensor.matmul(ps_r[:], Jt[:], rF, start=True, stop=True)
            out2 = wpool.tile([NT, 2, BPC, NP], fp32, tag="out2")
            inter = out2[:, 1]
            nc.vector.tensor_mul(inter, f, ps_r[:])
            fr = wpool.tile([NT, BPC, NP], fp32, tag="fr")
            nc.vector.tensor_add(fr[:], f, ps_r[:])
            ov = out_d[:].rearrange("o b (t p) -> o t b p", t=NT)
            nc.scalar.dma_start(ov[1], inter)
            nc.vector.scalar_tensor_tensor(
                out2[:, 0], inter, sig_w, fr[:], op0=OP.mult, op1=OP.add)
            nc.sync.dma_start(ov[0], out2[:, 0])

    nc.finalize()
    return nc


def _build_program_v1(c1, w, sig_w):
    """Per-direction fallback program (handles c1f != c1r or wf != wr)."""
    import concourse.bacc as bacc
    import concourse.mybir as mybir
    from concourse.tile import TileContext

    fp32 = mybir.dt.float32
    i32 = mybir.dt.int32
    mm_dt = fp32
    OP = mybir.AluOpType

    nc = bacc.Bacc(None, target_bir_lowering=False)

    GE = 2 * BPC * NP
    XW = BPC * NH
    CW = GE + XW
    consts_d = nc.dram_tensor("consts", [NT, CW], fp32, kind="ExternalInput")
    out_d = nc.dram_tensor("out_all", [2, BPC, N], fp32, kind="ExternalOutput")

    (c1f, c1r), (wf, wr) = c1, w

    g_off = 0
    x0_off = g_off + GE

    with TileContext(nc) as tc:
        with (
            tc.tile_pool(name="const", bufs=1) as cpool,
            tc.tile_pool(name="state", bufs=3) as spool,
            tc.tile_pool(name="work", bufs=3) as wpool,
            tc.tile_pool(name="accp", bufs=2) as apool,
            tc.tile_pool(name="psum", bufs=4, space="PSUM") as ppool,
        ):
            consts = cpool.tile([NT, CW], fp32, tag="consts")
            xh = XW // 2
            nc.scalar.dma_start(consts[:, x0_off:x0_off + xh],
                                consts_d[:, x0_off:x0_off + xh])
            nc.sync.dma_start(consts[:, x0_off + xh:x0_off + XW],
                              consts_d[:, x0_off + xh:x0_off + XW])
            nc.sync.dma_start(consts[:, g_off:g_off + GE],
                              consts_d[:, g_off:g_off + GE])

            matsf = cpool.tile([NT, 4 * NT], fp32, tag="matsf")
            vf = cpool.tile([NT, NT], i32, tag="vf")
            vr = cpool.tile([NT, NT], i32, tag="vr")
            nc.gpsimd.iota(vf[:], pattern=[[1, NT]], base=NT,
                           channel_multiplier=-1)
            nc.gpsimd.iota(vr[:], pattern=[[-1, NT]], base=NT,
                           channel_multiplier=1)
            nc.vector.tensor_scalar(vf[:], vf[:], scalar1=NT - 1, scalar2=None,
                                    op0=OP.bitwise_and)
            nc.vector.tensor_scalar(vr[:], vr[:], scalar1=NT - 1, scalar2=None,
                                    op0=OP.bitwise_and)
            nc.vector.tensor_scalar(matsf[:, 0:NT], vf[:], scalar1=1,
                                    scalar2=None, op0=OP.is_equal)
            nc.vector.tensor_scalar(matsf[:, 2 * NT:3 * NT], vr[:], scalar1=1,
                                    scalar2=None, op0=OP.is_equal)
            nc.vector.tensor_scalar(matsf[:, NT:2 * NT], vf[:], scalar1=2,
                                    scalar2=None, op0=OP.is_lt)
            nc.vector.tensor_scalar(matsf[:, 3 * NT:4 * NT], vr[:], scalar1=2,
                                    scalar2=None, op0=OP.is_lt)
            pm = {k: matsf[:, i * NT:(i + 1) * NT].bitcast(mm_dt)
                  for i, k in enumerate(("Pf", "Mf", "Pr", "Mr"))}
            g_e = consts[:, g_off:g_off + GE].rearrange(
                "t (d b p) -> t d b p", d=2, b=BPC)

            x0 = consts[:, x0_off:x0_off + XW].rearrange(
                "t (b p) -> t b p", b=BPC)

            DIRS = (
                dict(d=0, P="Pf", M="Mf", c1=c1f, w=wf, xtag="xf"),
                dict(d=1, P="Pr", M="Mr", c1=c1r, w=wr, xtag="xr"),
            )
            xs = [x0, x0]
            accs = [None, None]
            for s in range(S):
                xcs = [None, None]
                for dd in DIRS:
                    d = dd["d"]
                    xd = xs[d]
                    ps = ppool.tile([NT, BPC, NP], fp32, tag=f"ps{d}")
                    xm = xd.bitcast(mm_dt)
                    lo = 0 if d == 0 else 2
                    nc.tensor.matmul(ps[:], pm[dd["P"]], xm[:, :, 1:NP + 1],
                                     start=True, stop=False)
                    nc.tensor.matmul(ps[:], pm[dd["M"]], xm[:, :, lo:lo + NP],
                                     start=False, stop=True)

                    tmp = wpool.tile([NT, BPC, NP], fp32, tag=f"tmp{d}")
                    nc.vector.tensor_mul(tmp[:], ps[:], g_e[:, d])
                    xn = spool.tile([NT, BPC, NH], fp32, tag=dd["xtag"])
                    xcs[d] = xn[:, :, 1:NP + 1]
                    nc.vector.scalar_tensor_tensor(
                        xcs[d], xd[:, :, 1:NP + 1], dd["c1"], tmp[:],
                        op0=OP.mult, op1=OP.add)
                    if s < S - 1:
                        nc.gpsimd.tensor_copy(xn[:, :, 0:1], xn[:, :, NP:NP + 1])
                        nc.gpsimd.tensor_copy(xn[:, :, NP + 1:NP + 2], xn[:, :, 1:2])
                    xs[d] = xn[:]

                for dd in DIRS:
                    d = dd["d"]
                    an = apool.tile([NT, BPC, NP], fp32, tag=f"acc{d}")
                    if accs[d] is None:
                        nc.vector.tensor_scalar_mul(an[:], xcs[d], dd["w"][0])
                    else:
                        nc.vector.scalar_tensor_tensor(
                            an[:], xcs[d], dd["w"][s], accs[d][:],
                            op0=OP.mult, op1=OP.add)
                    accs[d] = an
            accs = [accs[0][:], accs[1][:]]

            out2 = wpool.tile([NT, 2, BPC, NP], fp32, tag="out2")
            inter = out2[:, 1]
            nc.vector.tensor_mul(inter, accs[0], accs[1])
            fr = wpool.tile([NT, BPC, NP], fp32, tag="fr")
            nc.vector.tensor_add(fr[:], accs[0], accs[1])
            ov = out_d[:].rearrange("o b (t p) -> o t b p", t=NT)
            nc.scalar.dma_start(ov[1], inter)
            nc.vector.scalar_tensor_tensor(
                out2[:, 0], inter, sig_w, fr[:], op0=OP.mult, op1=OP.add)
            nc.sync.dma_start(ov[0], out2[:, 0])

    nc.finalize()
    return nc


def _host_prep(inputs):
    entry = np.ascontiguousarray(np.asarray(inputs["entry_probs"], np.float32))
    fwd_adj = np.asarray(inputs["forward_adj"], np.float32)
    rev_adj = np.asarray(inputs["reverse_adj"], np.float32)
    angles = np.asarray(inputs["bounce_angles"], np.float32)

    vf = _diag_vals(fwd_adj, _FWD)
    vr = _diag_vals(rev_adj, _REV)
    ok = _structure_ok(fwd_adj, vf) and _structure_ok(rev_adj, vr)

    df = float(np.clip(float(np.asarray(inputs["forward_decay"])), 0.5, 0.99))
    dr = float(np.clip(float(np.asarray(inputs["reverse_decay"])), 0.5, 0.99))
    wf = _softmax(np.asarray(inputs["forward_step_weights"], np.float32))
    wr = _softmax(np.asarray(inputs["reverse_step_weights"], np.float32))
    sig = float(1.0 / (1.0 + np.exp(-float(np.asarray(inputs["interaction_weight"])))))

    vbf = [float(v.mean()) for v in vf]   # [v10, v01, v11]
    vbr = [float(v.mean()) for v in vr]
    # 0/1 matrices require one shared constant per direction
    for vs in (vbf, vbr):
        if abs(vs[0] - vs[1]) > 1e-6 * abs(vs[0]) or \
           abs(vs[0] - vs[2]) > 1e-6 * abs(vs[0]):
            ok = False

    af = (0.5 + 0.5 * np.cos(np.abs(angles).mean(axis=1))).astype(np.float32)
    af2 = af.reshape(NT, NP)

    c1f, c1r = 0.3 * df, 0.3 * dr
    fused = ok and abs(c1f - c1r) < 1e-12 and \
        all(abs(a - b) < 1e-12 for a, b in zip(wf, wr))

    consts_list = []
    x0_list = []
    if fused:
        # g[t, d, p]; reverse half theta-flipped
        g = np.empty((NT, 2, NP), np.float32)
        g[:, 0] = (0.7 * df * vbf[0]) * af2
        g[:, 1] = (0.7 * dr * vbr[0]) * af2[::-1]
        gflat = np.ascontiguousarray(g.reshape(NT, -1))
        for c in range(NCORES):
            e = entry[c * BPC:(c + 1) * BPC].reshape(BPC, NT, NP)
            x0 = np.empty((NT, 2, BPC, NH), np.float32)
            x0[:, 0, :, 1:NP + 1] = e.transpose(1, 0, 2)
            x0[:, 1, :, 1:NP + 1] = e[:, ::-1].transpose(1, 0, 2)
            x0[:, :, :, 0] = x0[:, :, :, NP]
            x0[:, :, :, NP + 1] = x0[:, :, :, 1]
            consts_list.append(gflat)
            x0_list.append(np.ascontiguousarray(
                x0.reshape(NT, -1).astype(np.float16)))
    elif ok:
        g_e = np.empty((NT, 2, BPC, NP), np.float32)
        g_e[:, 0] = (0.7 * df * vbf[0]) * af2[:, None, :]
        g_e[:, 1] = (0.7 * dr * vbr[0]) * af2[:, None, :]
        consts_common = g_e.reshape(NT, -1)
        for c in range(NCORES):
            e = entry[c * BPC:(c + 1) * BPC].reshape(BPC, NT, NP).transpose(1, 0, 2)
            x0 = np.empty((NT, BPC, NH), np.float32)
            x0[:, :, 1:NP + 1] = e
            x0[:, :, 0] = e[:, :, NP - 1]
            x0[:, :, NP + 1] = e[:, :, 0]
            consts_list.append(np.ascontiguousarray(
                np.concatenate([consts_common, x0.reshape(NT, -1)], axis=1)))

    meta = dict(
        ok=ok, fused=fused,
        c1=(c1f, c1r),
        w=(list(map(float, wf)), list(map(float, wr))),
        sig=sig, consts_list=consts_list, x0_list=x0_list,
    )
    return meta


_PROGRAM_CACHE = {}
LAST_RESULT = None


def kernel(**inputs):
    meta = _host_prep(inputs)
    if not meta["ok"]:
        return _reference_fallback(
            np.asarray(inputs["entry_probs"], np.float32),
            np.asarray(inputs["forward_adj"], np.float32),
            np.asarray(inputs["reverse_adj"], np.float32),
            inputs["forward_step_weights"], inputs["forward_decay"],
            inputs["reverse_step_weights"], inputs["reverse_decay"],
            inputs["interaction_weight"], np.asarray(inputs["bounce_angles"], np.float32))

    # If tracing is requested via BASS_TRACE but the image's antenv lacks
    # axon_hooks, provide the hook so run_bass_kernel_spmd doesn't crash.
    import os as _os
    if _os.environ.get("BASS_TRACE"):
        try:
            import antenv.axon_hooks  # noqa: F401
        except ImportError:
            try:
                import sys as _sys
                import types as _types
                import trn_agent_boot.trn_boot as _tb
                _hook = _tb._ntff_profile_via_ctypes("/opt/axon/libaxon_pjrt.so")
                _mod = _types.ModuleType("antenv.axon_hooks")
                _mod.get_axon_ntff_profile_hook = lambda: _hook
                _mod.set_axon_ntff_profile_hook = lambda h: None
                _sys.modules["antenv.axon_hooks"] = _mod
            except Exception:
                _os.environ.pop("BASS_TRACE", None)

    from concourse import bass_utils

    key = (tuple(meta["c1"]), tuple(meta["w"][0]), tuple(meta["w"][1]),
           meta["sig"], meta["fused"])
    if key not in _PROGRAM_CACHE:
        if meta["fused"]:
            _PROGRAM_CACHE[key] = _build_program_v2(
                meta["c1"][0], meta["w"][0], meta["sig"])
        else:
            _PROGRAM_CACHE[key] = _build_program_v1(
                meta["c1"], meta["w"], meta["sig"])
    nc = _PROGRAM_CACHE[key]

    if meta["fused"]:
        in_maps = [{"consts": meta["consts_list"][c], "x0": meta["x0_list"][c]}
                   for c in range(NCORES)]
    else:
        in_maps = [{"consts": meta["consts_list"][c]} for c in range(NCORES)]
    res = bass_utils.run_bass_kernel_spmd(nc, in_maps, core_ids=list(range(NCORES)))
    global LAST_RESULT
    LAST_RESULT = res

    combined = np.concatenate([r["out_all"][0] for r in res.results], axis=0)
    interaction = np.concatenate([r["out_all"][1] for r in res.results], axis=0)
    return combined, interaction
